# revision 4
# baseline (speedup 1.0000x reference)
"""Trainium2 Bass kernel for LogitBiasedSelfAttention1D.

Sharding: 8 cores = (batch b in 0..3) x (query half qh in 0..1).
Each core computes full attention (all 8 heads, all 2048 keys) for the
1024 queries of its batch half. No collectives.

Math decomposition (exactly equivalent to the reference up to fp):
  - conv1d key bias folded into exp:  softmax(S + bias) via the Act
    engine's per-partition activation bias (bias = key-indexed AP), and
    on the DVE via the Schraudolph bits constant.  V carries a 65th
    all-ones column per head so PV also produces the softmax
    denominators.
  - SCALE folded into w_q on host.
  - b_out + residual x_seq folded into one host-prepared addend.
  - LayerNorm gamma/beta folded into the final transpose drain.

Engines:
  - PE: all matmuls.  QKV / out-proj use fp8e4 DoubleRow (2 contraction
    planes per pass); PV uses fp8-DR for the Act-exp'd query columns and
    plain bf16 for the DVE-exp'd columns.  S stays bf16.
  - Act: exact exp (fp8e4 out) for NA of the 1024 query columns per
    (chunk, head-pair-half).
  - DVE: Schraudolph int16-bits exp (bf16 via bitcast) for the rest,
    plus all PSUM drains, PV normalize (broadcast tensor_tensor), and
    the out-proj accumulate.
  - Pool: LN tail scalar work.
"""

import sys

for _p in ("/opt/trn_rl_repo", "/root/.axon_site/_ro/trn_rl_repo"):
    if _p not in sys.path:
        sys.path.insert(0, _p)

import numpy as np
import ml_dtypes

from concourse import bass, mybir
from concourse.tile import TileContext
from concourse.bass_utils import run_bass_kernel_spmd

B, C, T = 4, 512, 2048
H, D = 8, 64
SCALE = D ** -0.5
EPS = 1e-5
TQ = T // 2            # queries per core
KC = T // 128          # 16 key chunks
KK = KC // 2           # 8 chunk pairs
PAIRS = H // 2         # 4 head pairs
F32 = mybir.dt.float32
BF16 = mybir.dt.bfloat16
FP8 = mybir.dt.float8e4
I16 = mybir.dt.int16
bf16 = ml_dtypes.bfloat16
fp8 = ml_dtypes.float8_e4m3

Exp = mybir.ActivationFunctionType.Exp
Sqrt = mybir.ActivationFunctionType.Sqrt
Square = mybir.ActivationFunctionType.Square
Ident = mybir.ActivationFunctionType.Identity
MULT = mybir.AluOpType.mult
ADD = mybir.AluOpType.add
DR = mybir.MatmulPerfMode.DoubleRow

LOG2E = 1.4426950408889634
A16 = 128.0 * LOG2E          # Schraudolph slope (bf16 bits)
C_ADJ = -128.0 * 0.04305     # balanced max-rel-err constant

# Act-exp'd query columns per (chunk, hi); rest go to the DVE.
NA = (768, 640)

_CACHE = {}


def _bcol(b):
    """Column offset of 65-wide PV block b (0..15) in the 3-bank OC tile.
    7 + 7 + 2 blocks per bank; no block crosses a 512-col bank boundary.
    b = hi*8 + s."""
    if b < 7:
        return b * 65
    if b < 14:
        return 512 + (b - 7) * 65
    return 1024 + (b - 14) * 65


def _build_nc():
    nc = bass.Bass()
    # packed layouts: one DMA per logical tensor; [128, n*512] with the
    # 128-row blocks of the original (rows, cols) tensor side by side.
    # Token chunks are rotated per core so this core's query half is always
    # chunks j=0,1 (softmax is key-order invariant; the per-key bias is
    # rotated to match), so Q-projection reads XC directly.
    xct = nc.declare_dram_parameter("xct", [128, 4 * T], FP8, False)
    xseq = nc.declare_dram_parameter("xseq", [TQ, C], F32, False)
    wq = nc.declare_dram_parameter("wq", [128, 4 * C], FP8, False)
    wk = nc.declare_dram_parameter("wk", [128, 4 * C], FP8, False)
    wv = nc.declare_dram_parameter("wv", [128, 4 * C], FP8, False)
    wo = nc.declare_dram_parameter("wo", [128, 4 * C], FP8, False)
    abia = nc.declare_dram_parameter("abia", [128, KC], F32, False)
    sbia = nc.declare_dram_parameter("sbia", [128, KC], F32, False)
    gmm = nc.declare_dram_parameter("gmm", [128, 4], F32, False)
    bet = nc.declare_dram_parameter("bet", [128, 4], F32, False)
    iden = nc.declare_dram_parameter("iden", [128, 128], BF16, False)
    outp = nc.declare_dram_parameter("out", [C, TQ], F32, True)

    with TileContext(nc) as tc:
        with (
            tc.sbuf_pool(name="cst", bufs=1) as cst,
            tc.sbuf_pool(name="pex", bufs=3) as pex,
            tc.sbuf_pool(name="sml", bufs=2) as sml,
            tc.psum_pool(name="ps", bufs=1) as ps,
        ):
            # ---- critical-path constants, in DMA priority order ----
            ID = cst.tile_from(iden[:, :], name="ID")
            WKa = cst.tile_from(wk[:, 0:512], name="WKa")
            XC = [None] * 4
            XC[0] = cst.tile_from(xct[:, 0:2048], name="XCj0")
            WQa = cst.tile_from(wq[:, 0:512], name="WQa")
            XC[1] = cst.tile_from(xct[:, 2048:4096], name="XCj1")
            WKb = cst.tile_from(wk[:, 512:2048], name="WKb")
            WQb = cst.tile_from(wq[:, 512:2048], name="WQb")
            WV4 = cst.tile_from(wv[:, :], name="WV4")
            AB = cst.tile_from(abia[:, :], name="AB")
            SB16 = cst.tile_from(sbia[:, :], name="SB16")
            for j in range(2, 4):
                XC[j] = cst.tile_from(xct[:, j * 2048:(j + 1) * 2048],
                                      name=f"XCj{j}")
            WK = (WKa, WKb)
            WQ = (WQa, WQb)

            # PE p-state warmup: chain dummy transposes while the first
            # input DMAs stream in.
            warm = ps.tile([128, 128], BF16, tag="FA", name="warm")
            for _ in range(48):
                nc.tensor.transpose(warm[:, :], ID[:, :], ID[:, :])

            # ---- persistent SBUF tiles ----
            KT = [cst.tile([128, T], BF16, name=f"KT{m}") for m in range(4)]
            QT = [cst.tile([128, TQ], BF16, name=f"QT{m}") for m in range(4)]
            # VB2[kk]: [128, 2*(H*65)] fp8, plane i = chunk 2kk+i; the 65th
            # column of each head block is 1.0 (softmax denominator).
            VB2 = [cst.tile([128, 2 * H * 65], FP8, name=f"VB{k}")
                   for k in range(KK)]
            OT = cst.tile([128, 4 * TQ], FP8, name="OT")   # [128, pair, TQ]
            OACC = [cst.tile([128, C], F32, name=f"OACC{t}") for t in range(8)]

            for kk in range(KK):
                for i in range(2):
                    nc.gpsimd.memset(
                        VB2[kk][:, i * 520:(i + 1) * 520].rearrange(
                            "p (h e) -> p h e", e=65)[:, :, 64:65], 1.0 / 64)

            # ---- feeder machinery ----
            feeders = []
            done = set()

            def pump(n=1):
                for _ in range(n):
                    if feeders:
                        key, fn = feeders.pop(0)
                        fn()
                        done.add(key)

            def ensure(key):
                while key not in done:
                    assert feeders, f"missing feeder quantum {key}"
                    k2, fn = feeders.pop(0)
                    fn()
                    done.add(k2)

            def kq_quantum(dst, Wab, m, j, tag="FA", act_copy=False):
                # dst[:, j*512:(j+1)*512] = W[:, m-block].T @ x-cols-j
                # fp8 DoubleRow over ci-plane pairs.
                def emit():
                    W = Wab[0] if m == 0 else Wab[1]
                    c0 = (0 if m == 0 else (m - 1) * 512)
                    fps = ps.tile([128, 512], F32, tag=tag,
                                  name=f"f_{dst.tensor.name}_{j}")
                    for c2 in range(2):
                        nc.tensor.matmul(
                            fps[:, :],
                            lhsT=W[:, c0 + c2 * 256:c0 + (c2 + 1) * 256]
                                .rearrange("p (i m2) -> p i m2", i=2),
                            rhs=XC[j][:, c2 * 1024:(c2 + 1) * 1024]
                                .rearrange("p (i n) -> p i n", i=2),
                            start=(c2 == 0), stop=(c2 == 1),
                            perf_mode=DR)
                    if act_copy:
                        nc.scalar.copy(dst[:, j * 512:(j + 1) * 512], fps[:, :])
                    else:
                        nc.vector.tensor_copy(dst[:, j * 512:(j + 1) * 512],
                                              fps[:, :])
                return emit

            def v_quantum(k, tag="FA"):
                def emit():
                    fps = ps.tile([128, 512], F32, tag=tag, name=f"fv{k}")
                    for c2 in range(2):
                        nc.tensor.matmul(
                            fps[:, :],
                            lhsT=XC[k // 4].rearrange(
                                "p (c t) -> p c t", c=4)[
                                :, c2 * 2:(c2 + 1) * 2,
                                (k % 4) * 128:(k % 4) * 128 + 128],
                            rhs=WV4[:, c2 * 1024:(c2 + 1) * 1024]
                                .rearrange("p (i n) -> p i n", i=2),
                            start=(c2 == 0), stop=(c2 == 1),
                            perf_mode=DR)
                    i = k % 2
                    nc.vector.tensor_copy(
                        VB2[k // 2][:, i * 520:(i + 1) * 520].rearrange(
                            "p (h e) -> p h e", e=65)[:, :, 0:64],
                        fps.rearrange("p (h e) -> p h e", e=64))
                return emit

            # ---- prelude: just enough for pair 0 step 0 ----
            kq_quantum(KT[0], WK, 0, 0, "FA", act_copy=True)()
            kq_quantum(QT[0], WQ, 0, 0, "SA")()
            kq_quantum(QT[0], WQ, 0, 1, "SB", act_copy=True)()
            done.update({("K", 0, 0), ("Q", 0, 0), ("Q", 0, 1)})

            # ---- feeder queue ----
            feeders.append((("V", 0), v_quantum(0)))
            feeders.append((("V", 1), v_quantum(1)))
            feeders.append((("K", 0, 1), kq_quantum(KT[0], WK, 0, 1)))
            feeders.append((("V", 2), v_quantum(2)))
            feeders.append((("V", 3), v_quantum(3)))
            feeders.append((("K", 0, 2), kq_quantum(KT[0], WK, 0, 2)))
            feeders.append((("V", 4), v_quantum(4)))
            feeders.append((("V", 5), v_quantum(5)))
            feeders.append((("K", 0, 3), kq_quantum(KT[0], WK, 0, 3)))
            for k in range(6, KC):
                feeders.append((("V", k), v_quantum(k)))
            for m in range(1, 4):
                feeders.append((("K", m, 0), kq_quantum(KT[m], WK, m, 0)))
                for j in range(2):
                    feeders.append((("Q", m, j),
                                    kq_quantum(QT[m], WQ, m, j)))
                for j in range(1, 4):
                    feeders.append((("K", m, j),
                                    kq_quantum(KT[m], WK, m, j)))

            # deferred epilogue inputs
            WO2 = cst.tile_from(wo[:, :], name="WO2")   # [128, pair*512+cout]
            XS = [cst.tile_from(xseq[t * 128:(t + 1) * 128, :], name=f"XS{t}")
                  for t in range(8)]
            GM = cst.tile_from(gmm[:, :], name="GM")
            BT = cst.tile_from(bet[:, :], name="BT")
            epsT = cst.tile([128, 1], F32, name="epsT")
            nc.vector.memset(epsT[:, :], EPS)

            # ---- attention ----
            def make_tp_quantum(p, ON, half):
                def emit():
                    tp = ps.tile([128, 512], BF16, tag="FA",
                                 name=f"tp{p}_{half}")
                    for j in range(4):
                        s = half * 4 + j
                        nc.tensor.transpose(
                            tp[:, j * 128:(j + 1) * 128],
                            ON[:, s * 128:(s + 1) * 128], ID[:, :])
                    nc.vector.tensor_copy(
                        OT[:, p * TQ + half * 512:p * TQ + (half + 1) * 512],
                        tp[:, :])
                return emit

            def make_boundary(p, oc):
                # normalize pair p: reciprocal of denominators, then one
                # broadcast tensor_tensor per contiguous oc-block group.
                # ON layout: [128, s*128 + hi*64] (s-major) so the
                # transposes read plain 128-col slices.
                def emit():
                    rd16 = sml.tile([128, 16], F32, tag="rd", name=f"rd{p}",
                                    bufs=2)
                    for lo, nblk, b0 in ((0, 7, 0), (512, 7, 7), (1024, 2, 14)):
                        nc.vector.reciprocal(
                            rd16[:, b0:b0 + nblk].rearrange(
                                "p (s e) -> p s e", e=1),
                            oc[:, lo:lo + nblk * 65].rearrange(
                                "p (s e) -> p s e", e=65)[:, :, 64:65])
                    ON = sml.tile([128, 1024], BF16, tag="on",
                                  name=f"on{p}", bufs=2)
                    ONb = ON.rearrange("p (q e) -> p q e", e=64)
                    # groups of oc blocks b = hi*8+s with uniform strides:
                    # (b0..b0+n) -> ON block index s*2+hi
                    for b0, nblk, lo in ((0, 7, 0), (7, 1, 512),
                                         (8, 6, 512 + 65), (14, 2, 1024)):
                        hi, s0 = divmod(b0, 8)
                        nc.vector.tensor_tensor(
                            out=ONb[:, 2 * s0 + hi:2 * (s0 + nblk - 1) + hi + 1:2, :],
                            in0=oc[:, lo:lo + nblk * 65].rearrange(
                                "p (s e) -> p s e", e=65)[:, :, 0:64],
                            in1=rd16[:, b0:b0 + nblk].rearrange(
                                "p (s e) -> p s e", e=1).to_broadcast(
                                [128, nblk, 64]),
                            op=MULT)
                    feeders.insert(0, (("TP", p, 1), make_tp_quantum(p, ON, 1)))
                    feeders.insert(0, (("TP", p, 0), make_tp_quantum(p, ON, 0)))
                return emit

            pending_boundary = None
            for p in range(PAIRS):
                oc = ps.tile([128, 1536], F32, tag="OC", name=f"oc{p}")

                def pv_emit(kk, pts, oc=oc, p=p):
                    # pts = {hi: (PT8, PTB)}
                    for hi in range(2):
                        head = 2 * p + hi
                        pt8, ptb = pts[hi]
                        na = NA[hi]
                        rhs2 = VB2[kk].rearrange("p (i c) -> p i c", i=2)[
                            :, :, head * 65:(head + 1) * 65]
                        for s in range(na // 128):
                            nc.tensor.matmul(
                                oc[:, _bcol(hi * 8 + s):_bcol(hi * 8 + s) + 65],
                                lhsT=pt8.rearrange("p (i q) -> p i q", i=2)[
                                    :, :, s * 128:(s + 1) * 128],
                                rhs=rhs2,
                                start=(kk == 0), stop=(kk == KK - 1),
                                perf_mode=DR)
                        for s in range(na // 128, 8):
                            col = _bcol(hi * 8 + s)
                            q0 = s * 128 - na
                            for i in range(2):
                                nc.tensor.matmul(
                                    oc[:, col:col + 65],
                                    lhsT=ptb[:, i * (1024 - na) + q0:
                                             i * (1024 - na) + q0 + 128]
                                        .bitcast(BF16),
                                    rhs=VB2[kk][:, i * 520 + head * 65:
                                                i * 520 + (head + 1) * 65],
                                    start=(kk == 0 and i == 0),
                                    stop=(kk == KK - 1 and i == 1))

                ensure(("Q", p, 0))
                ensure(("Q", p, 1))
                lag = []      # chunk-pair PV deferral
                pts_cur = {}
                for k in range(KC):
                    ensure(("K", p, k // 4))
                    kk = k // 2
                    if k % 2 == 0:
                        pts_cur = {
                            hi: (pex.tile([128, 2 * NA[hi]], FP8,
                                          tag=f"p8{hi}", name=f"p8_{p}_{kk}_{hi}"),
                                 pex.tile([128, 2 * (1024 - NA[hi])], I16,
                                          tag=f"pb{hi}", name=f"pb_{p}_{kk}_{hi}"))
                            for hi in range(2)}
                    for hi in range(2):
                        rows = slice(hi * 64, (hi + 1) * 64)
                        na = NA[hi]
                        s_ps = ps.tile([128, 1024], F32, tag=("SA", "SB")[hi],
                                       name=f"s{p}_{k}_{hi}")
                        for n in range(2):
                            nc.tensor.matmul(
                                s_ps[:, n * 512:(n + 1) * 512],
                                lhsT=KT[p][rows, k * 128:(k + 1) * 128],
                                rhs=QT[p][rows, n * 512:(n + 1) * 512],
                                start=True, stop=True)
                        pt8, ptb = pts_cur[hi]
                        i = k % 2
                        nc.scalar.activation(
                            pt8[:, i * na:(i + 1) * na], s_ps[:, 0:na], Exp,
                            bias=AB[:, k:k + 1], scale=1.0)
                        nc.vector.tensor_scalar(
                            out=ptb[:, i * (1024 - na):(i + 1) * (1024 - na)],
                            in0=s_ps[:, na:1024], scalar1=A16,
                            scalar2=SB16[:, k:k + 1], op0=MULT, op1=ADD)
                    if k == 0 and pending_boundary is not None:
                        pending_boundary()
                        pending_boundary = None
                    if k % 2 == 1:
                        lag.append((kk, pts_cur))
                    # drain the PV lag progressively near the pair end
                    depth = 3 if k < KC - 3 else (2 if k < KC - 1 else 1)
                    while len(lag) > depth:
                        kkd, pp = lag.pop(0)
                        ensure(("V", 2 * kkd))
                        ensure(("V", 2 * kkd + 1))
                        pv_emit(kkd, pp)
                    if p == 0:
                        if k < KC - 2:
                            pump(2 if k < 8 else 1)
                    elif k < KC - 2:
                        pump(2 if k < 2 else 1)
                for kkd, pp in lag:
                    ensure(("V", 2 * kkd))
                    ensure(("V", 2 * kkd + 1))
                    pv_emit(kkd, pp)
                pending_boundary = make_boundary(p, oc)

            pending_boundary()
            ensure(("TP", 3, 0))

            # ---- tail: out-proj (fp8-free bf16 DR over pair planes) + LN ----
            nmrs, rstds, accs = [], [], {}

            def tail_o(t):
                ensure(("TP", 3, t // 4))
                tag = ("SA", "SB", "FA")[t % 3]
                fps = ps.tile([128, 512], F32, tag=tag, name=f"fo{t}")
                for j in range(2):
                    nc.tensor.matmul(
                        fps[:, :],
                        lhsT=OT.rearrange("p (q4 q) -> p q4 q", q4=4)[
                            :, 2 * j:2 * j + 2, t * 128:(t + 1) * 128],
                        rhs=WO2[:, j * 1024:(j + 1) * 1024]
                            .rearrange("p (i n) -> p i n", i=2),
                        start=(j == 0), stop=(j == 1),
                        perf_mode=DR)
                acc = sml.tile([128, 1], F32, tag="acc", name=f"acc{t}", bufs=4)
                accs[t] = acc
                nc.vector.scalar_tensor_tensor(
                    out=OACC[t], in0=fps[:, :], scalar=1.0 / 1024, in1=XS[t],
                    op0=MULT, op1=ADD, accum_out=acc)

            def tail_a(t):
                sq = sml.tile([128, C], F32, tag="sq", name=f"sq{t}", bufs=2)
                ssq = sml.tile([128, 1], F32, tag="ssq", name=f"ssq{t}", bufs=3)
                nc.scalar.activation(sq[:, :], OACC[t][:, :], Square,
                                     accum_out=ssq[:, :])
                dvar = sml.tile([128, 1], F32, tag="dvar", name=f"dv{t}", bufs=3)
                nc.gpsimd.tensor_scalar(out=dvar[:, :], in0=accs[t],
                                        scalar1=accs[t], scalar2=-1.0 / C,
                                        op0=MULT, op1=MULT)
                nc.vector.scalar_tensor_tensor(
                    out=dvar[:, :], in0=dvar[:, :], scalar=1.0,
                    in1=ssq[:, :], op0=MULT, op1=ADD)
                std = sml.tile([128, 1], F32, tag="std", name=f"std{t}", bufs=3)
                nc.scalar.activation(std[:, :], dvar[:, :], Sqrt,
                                     bias=epsT[:, :], scale=1.0 / C)
                rstd = sml.tile([128, 1], F32, tag="rstd", name=f"rstd{t}", bufs=3)
                nc.vector.reciprocal(rstd[:, :], std[:, :])
                nmr = sml.tile([128, 1], F32, tag="nmr", name=f"nmr{t}", bufs=3)
                nc.gpsimd.tensor_scalar(out=nmr[:, :], in0=accs[t],
                                        scalar1=rstd[:, :], scalar2=-1.0 / C,
                                        op0=MULT, op1=MULT)
                rstds.append(rstd)
                nmrs.append(nmr)

            def tail_b(t):
                hn = sml.tile([128, C], BF16, tag="hn", name=f"hn{t}", bufs=3)
                nc.gpsimd.tensor_scalar(out=hn[:, :], in0=OACC[t][:, :],
                                        scalar1=rstds[t], scalar2=nmrs[t],
                                        op0=MULT, op1=ADD)
                fa = ps.tile([128, 256], BF16, tag="OC", name=f"ftpa{t}")
                fb = ps.tile([128, 256], BF16, tag="FA", name=f"ftpb{t}")
                for cc in range(4):
                    dst = fa if cc < 2 else fb
                    nc.tensor.transpose(
                        dst[:, (cc % 2) * 128:(cc % 2) * 128 + 128],
                        hn[:, cc * 128:(cc + 1) * 128], ID[:, :])
                oa = sml.tile([128, 256], F32, tag="outa", name=f"outa{t}", bufs=2)
                ob = sml.tile([128, 256], F32, tag="outb", name=f"outb{t}", bufs=2)
                for cc in range(2):
                    nc.scalar.activation(
                        oa[:, cc * 128:(cc + 1) * 128],
                        fa[:, cc * 128:(cc + 1) * 128],
                        Ident, bias=BT[:, cc:cc + 1], scale=GM[:, cc:cc + 1])
                for cc in range(2, 4):
                    nc.vector.tensor_scalar(
                        out=ob[:, (cc - 2) * 128:(cc - 1) * 128],
                        in0=fb[:, (cc - 2) * 128:(cc - 1) * 128],
                        scalar1=GM[:, cc:cc + 1], scalar2=BT[:, cc:cc + 1],
                        op0=MULT, op1=ADD)
                nc.sync.dma_start(
                    out=outp[0:256, t * 128:(t + 1) * 128].rearrange(
                        "(c p) q -> p c q", p=128),
                    in_=oa.rearrange("p (c q) -> p c q", c=2))
                nc.sync.dma_start(
                    out=outp[256:512, t * 128:(t + 1) * 128].rearrange(
                        "(c p) q -> p c q", p=128),
                    in_=ob.rearrange("p (c q) -> p c q", c=2))

            for t in range(8):
                tail_o(t)
                tail_a(t)
                if t >= 1:
                    tail_b(t - 1)
            tail_b(7)

    _split_mm_waits(nc)
    return nc


def _split_mm_waits(nc):
    """Walrus MM structs carry only one sync wait; move extras to a NoOp."""
    f = nc.m.functions[0]
    for bb in f.blocks:
        il = bb.instructions
        out, changed = [], False
        for i in il:
            si = getattr(i, "sync_info", None)
            tn = type(i).__name__
            splittable = tn.startswith("Inst") and tn not in ("InstNoOp", "InstAllEngineBarrier")
            if (splittable and si is not None
                    and si.on_wait is not None and len(si.on_wait) > 1):
                waits = list(si.on_wait)
                for wi, w in enumerate(waits[:-1]):
                    out.append(mybir.InstNoOp(
                        name=f"{i.name}-wsplit{wi}", engine=i.engine,
                        sync_info=mybir.SyncInfo(on_wait=[w], on_update=[])))
                i.sync_info = mybir.SyncInfo(
                    on_wait=[waits[-1]], on_update=list(si.on_update))
                changed = True
            out.append(i)
        if changed:
            bb.instructions = out


def _prep_inputs(x, sqi, w_qkv, w_out, b_out, w_conv, b_conv, ln_gamma, ln_beta):
    x = np.asarray(x, np.float32)
    sqi = np.asarray(sqi, np.float32)
    w_qkv = np.asarray(w_qkv, np.float32)
    w_out = np.asarray(w_out, np.float32)
    b_out = np.asarray(b_out, np.float32)
    w_conv = np.asarray(w_conv, np.float32)
    b_conv = np.asarray(b_conv, np.float32)
    ln_gamma = np.asarray(ln_gamma, np.float32)
    ln_beta = np.asarray(ln_beta, np.float32)

    sp = np.pad(sqi, ((0, 0), (1, 1)))
    bias = (w_conv[0] * sp[:, :-2] + w_conv[1] * sp[:, 1:-1]
            + w_conv[2] * sp[:, 2:] + b_conv)                    # (B, T)

    def pack4(a):
        """(512, n) -> [128, 4*n]: 128-row blocks side by side (one DMA)."""
        n = a.shape[1]
        return a.reshape(4, 128, n).transpose(1, 0, 2).reshape(128, 4 * n)

    def pack_mm(a):
        """(512, 512) -> [128, m*512 + ci*128 + c]: m-major so the m=0
        quarter is a contiguous prefix (separately-DMA'd tile)."""
        return a.reshape(4, 128, 4, 128).transpose(1, 2, 0, 3).reshape(128, 2048)

    wqT = pack_mm(w_qkv[:C].T * SCALE).astype(fp8)
    wkT = pack_mm(w_qkv[C:2 * C].T).astype(fp8)
    wvT = pack4(w_qkv[2 * C:].T).astype(fp8)
    woT = pack4(w_out.T * 16.0).astype(fp8)
    gm = ln_gamma.reshape(4, 128).T.copy().astype(np.float32)
    bt = ln_beta.reshape(4, 128).T.copy().astype(np.float32)
    iden = np.eye(128, dtype=bf16)

    in_maps = []
    for core in range(8):
        b, qh = divmod(core, 2)
        qs = slice(qh * TQ, (qh + 1) * TQ)
        # rotate tokens so this core's query half is chunks j=0,1
        xr = np.roll(x[b], -qh * TQ, axis=1)
        br = np.roll(bias[b], -qh * TQ)
        ab = br.reshape(KC, 128).T.copy().astype(np.float32)
        sb = (128.0 * (127.0 + LOG2E * br) + C_ADJ).reshape(
            KC, 128).T.copy().astype(np.float32)
        xp = np.concatenate(
            [pack4(xr[:, j * 512:(j + 1) * 512]) for j in range(4)], axis=1)
        in_maps.append({
            "xct": xp.astype(fp8),
            "xseq": (x[b].T[qs] + b_out).copy().astype(np.float32),
            "wq": wqT, "wk": wkT, "wv": wvT, "wo": woT,
            "abia": ab, "sbia": sb, "gmm": gm, "bet": bt, "iden": iden,
        })
    return in_maps


def kernel(x, sqi, w_qkv, w_out, b_out, w_conv, b_conv, ln_gamma, ln_beta,
           _trace=False):
    if "nc" not in _CACHE:
        _CACHE["nc"] = _build_nc()
    nc = _CACHE["nc"]
    in_maps = _prep_inputs(x, sqi, w_qkv, w_out, b_out, w_conv, b_conv,
                           ln_gamma, ln_beta)
    res = run_bass_kernel_spmd(nc, in_maps, core_ids=list(range(8)), trace=_trace)
    _CACHE["last_result"] = res
    out = np.empty((B, C, T), np.float32)
    for core in range(8):
        b, qh = divmod(core, 2)
        out[b][:, qh * TQ:(qh + 1) * TQ] = res.results[core]["out"]
    return out


# revision 15
# speedup vs baseline: 1.2521x; 1.2521x over previous
"""Trainium2 Bass kernel for LogitBiasedSelfAttention1D.

Sharding: 8 cores = (batch b in 0..3) x (query half qh in 0..1).
Each core computes full attention (all 8 heads, all 2048 keys) for the
1024 queries of its batch half. No collectives.

Math decomposition (exactly equivalent to the reference up to fp):
  - conv1d key bias folded into exp:  softmax(S + bias) via the Act
    engine's per-partition activation bias (bias = key-indexed AP), and
    on the DVE via the Schraudolph bits constant.  V carries a 65th
    all-ones column per head so PV also produces the softmax
    denominators.
  - SCALE folded into w_q on host.
  - b_out + residual x_seq folded into one host-prepared addend.
  - LayerNorm gamma/beta folded into the final transpose drain.

Engines:
  - PE: all matmuls.  QKV / out-proj use fp8e4 DoubleRow (2 contraction
    planes per pass); PV uses fp8-DR for the Act-exp'd query columns and
    plain bf16 for the DVE-exp'd columns.  S stays bf16.
  - Act: exact exp (fp8e4 out) for NA of the 1024 query columns per
    (chunk, head-pair-half).
  - DVE: Schraudolph int16-bits exp (bf16 via bitcast) for the rest,
    plus all PSUM drains, PV normalize (broadcast tensor_tensor), and
    the out-proj accumulate.
  - Pool: LN tail scalar work.
"""

import sys

for _p in ("/opt/trn_rl_repo", "/root/.axon_site/_ro/trn_rl_repo"):
    if _p not in sys.path:
        sys.path.insert(0, _p)

import numpy as np
import ml_dtypes

from concourse import bass, mybir
from concourse.tile import TileContext
from concourse.bass_utils import run_bass_kernel_spmd

B, C, T = 4, 512, 2048
H, D = 8, 64
SCALE = D ** -0.5
EPS = 1e-5
TQ = T // 2            # queries per core
KC = T // 128          # 16 key chunks
KK = KC // 2           # 8 chunk pairs
PAIRS = H // 2         # 4 head pairs
F32 = mybir.dt.float32
BF16 = mybir.dt.bfloat16
FP8 = mybir.dt.float8e4
I16 = mybir.dt.int16
bf16 = ml_dtypes.bfloat16
fp8 = ml_dtypes.float8_e4m3

Exp = mybir.ActivationFunctionType.Exp
Sqrt = mybir.ActivationFunctionType.Sqrt
Square = mybir.ActivationFunctionType.Square
Ident = mybir.ActivationFunctionType.Identity
MULT = mybir.AluOpType.mult
ADD = mybir.AluOpType.add
DR = mybir.MatmulPerfMode.DoubleRow

LOG2E = 1.4426950408889634
A16 = 128.0 * LOG2E          # Schraudolph slope (bf16 bits)
C_ADJ = -128.0 * 0.04305     # balanced max-rel-err constant

# Query-column split: the n=0 half of each S tile is exp'd by Act, the
# n=1 half by the DVE (Schraudolph).  Separate PSUM tiles per engine --
# the tile framework serializes cross-engine co-readers of one tile.
NA = 512

_CACHE = {}


def _bcol(b):
    """Column offset of 65-wide PV block b (0..15) in the 3-bank OC tile.
    7 + 7 + 2 blocks per bank; no block crosses a 512-col bank boundary.
    b = hi*8 + s."""
    if b < 7:
        return b * 65
    if b < 14:
        return 512 + (b - 7) * 65
    return 1024 + (b - 14) * 65


def _build_nc():
    nc = bass.Bass()
    # packed layouts: one DMA per logical tensor; [128, n*512] with the
    # 128-row blocks of the original (rows, cols) tensor side by side.
    # Token chunks are rotated per core so this core's query half is always
    # chunks j=0,1 (softmax is key-order invariant; the per-key bias is
    # rotated to match), so Q-projection reads XC directly.
    xct = nc.declare_dram_parameter("xct", [128, 4 * T], FP8, False)
    xseq = nc.declare_dram_parameter("xseq", [TQ, C], F32, False)
    wq = nc.declare_dram_parameter("wq", [128, 4 * C], FP8, False)
    wk = nc.declare_dram_parameter("wk", [128, 4 * C], FP8, False)
    wv = nc.declare_dram_parameter("wv", [128, 4 * C], FP8, False)
    wo = nc.declare_dram_parameter("wo", [128, 4 * C], FP8, False)
    abia = nc.declare_dram_parameter("abia", [128, KC], F32, False)
    sbia = nc.declare_dram_parameter("sbia", [128, KC], F32, False)
    gmm = nc.declare_dram_parameter("gmm", [128, 4], F32, False)
    bet = nc.declare_dram_parameter("bet", [128, 4], F32, False)
    iden = nc.declare_dram_parameter("iden", [128, 128], BF16, False)
    outp = nc.declare_dram_parameter("out", [C, TQ], F32, True)

    with TileContext(nc) as tc:
        with (
            tc.sbuf_pool(name="cst", bufs=1) as cst,
            tc.sbuf_pool(name="pex", bufs=5) as pex,
            tc.sbuf_pool(name="sml", bufs=2) as sml,
            tc.psum_pool(name="ps", bufs=1) as ps,
        ):
            # ---- critical-path constants, in DMA priority order ----
            ID = cst.tile_from(iden[:, :], name="ID")
            WKa = cst.tile_from(wk[:, 0:512], name="WKa")
            XC = [None] * 4
            XC[0] = cst.tile_from(xct[:, 0:2048], name="XCj0")
            WQa = cst.tile_from(wq[:, 0:512], name="WQa")
            XC[1] = cst.tile_from(xct[:, 2048:4096], name="XCj1")
            WKb = cst.tile_from(wk[:, 512:2048], name="WKb")
            WQb = cst.tile_from(wq[:, 512:2048], name="WQb")
            AB = cst.tile_from(abia[:, :], name="AB")
            SB16 = cst.tile_from(sbia[:, :], name="SB16")
            WV4 = cst.tile_from(wv[:, :], name="WV4")
            for j in range(2, 4):
                XC[j] = cst.tile_from(xct[:, j * 2048:(j + 1) * 2048],
                                      name=f"XCj{j}")
            WK = (WKa, WKb)
            WQ = (WQa, WQb)

            # PE p-state warmup: chain dummy transposes while the first
            # input DMAs stream in.
            warm = ps.tile([128, 128], BF16, tag="FA", name="warm")
            for _ in range(48):
                nc.tensor.transpose(warm[:, :], ID[:, :], ID[:, :])

            # ---- persistent SBUF tiles ----
            KT = [cst.tile([128, T], BF16, name=f"KT{m}") for m in range(4)]
            QT = [cst.tile([128, TQ], BF16, name=f"QT{m}") for m in range(4)]
            # VB2[kk]: [128, 2*(H*65)] fp8, plane i = chunk 2kk+i; the 65th
            # column of each head block is 1.0 (softmax denominator).
            VB2 = [cst.tile([128, 2 * H * 65], FP8, name=f"VB{k}")
                   for k in range(KK)]
            OT = cst.tile([128, 4 * TQ], FP8, name="OT")   # [128, pair, TQ]
            OACC = [cst.tile([128, C], F32, name=f"OACC{t}") for t in range(8)]

            for kk in range(KK):
                for i in range(2):
                    nc.gpsimd.memset(
                        VB2[kk][:, i * 520:(i + 1) * 520].rearrange(
                            "p (h e) -> p h e", e=65)[:, :, 64:65], 1.0 / 64)

            # ---- feeder machinery ----
            # Each quantum's emit() issues its PE matmuls and returns a
            # drain closure (PSUM -> SBUF copy).  The drain is deferred
            # until the NEXT quantum is pumped, so by the time it lands on
            # the DVE queue its matmul has long finished and it never
            # head-of-line-blocks the critical exp TSPs.
            feeders = []
            done = set()
            pending = []   # (key, drain_fn), at most 1 entry
            tp_pending = []  # [(key, fn)] transpose quanta, run mid-pair

            def _flush():
                while pending:
                    key, dfn = pending.pop(0)
                    dfn()
                    done.add(key)

            def pump(n=1):
                for _ in range(n):
                    if feeders:
                        _flush()
                        key, fn = feeders.pop(0)
                        d = fn()
                        if d is None:
                            done.add(key)
                        else:
                            pending.append((key, d))

            def run_tp(n=1):
                for _ in range(n):
                    if tp_pending:
                        key, fn = tp_pending.pop(0)
                        fn()
                        done.add(key)

            def ensure(key):
                while key not in done:
                    if any(k2 == key for k2, _ in pending):
                        _flush()
                        continue
                    if any(k2 == key for k2, _ in tp_pending):
                        run_tp()
                        continue
                    assert feeders, f"missing feeder quantum {key}"
                    _flush()
                    k2, fn = feeders.pop(0)
                    d = fn()
                    if d is not None:
                        d()
                    done.add(k2)

            _drain_rr = [0]

            def kq_quantum(dst, Wab, m, j, tag="FA", act_copy=None):
                # dst[:, j*512:(j+1)*512] = W[:, m-block].T @ x-cols-j
                # fp8 DoubleRow over ci-plane pairs.
                def emit():
                    W = Wab[0] if m == 0 else Wab[1]
                    c0 = (0 if m == 0 else (m - 1) * 512)
                    fps = ps.tile([128, 512], F32, tag=tag,
                                  name=f"f_{dst.tensor.name}_{j}")
                    for c2 in range(2):
                        nc.tensor.matmul(
                            fps[:, :],
                            lhsT=W[:, c0 + c2 * 256:c0 + (c2 + 1) * 256]
                                .rearrange("p (i m2) -> p i m2", i=2),
                            rhs=XC[j][:, c2 * 1024:(c2 + 1) * 1024]
                                .rearrange("p (i n) -> p i n", i=2),
                            start=(c2 == 0), stop=(c2 == 1),
                            perf_mode=DR)

                    if act_copy is None:
                        _drain_rr[0] = (_drain_rr[0] + 1) % 4
                        on_act = _drain_rr[0] != 0
                    else:
                        on_act = act_copy

                    def drain():
                        if on_act:
                            nc.scalar.copy(dst[:, j * 512:(j + 1) * 512],
                                           fps[:, :])
                        else:
                            nc.vector.tensor_copy(
                                dst[:, j * 512:(j + 1) * 512], fps[:, :])
                    return drain
                return emit

            def v_quantum(k, tag="FA"):
                def emit():
                    fps = ps.tile([128, 512], F32, tag=tag, name=f"fv{k}")
                    for c2 in range(2):
                        nc.tensor.matmul(
                            fps[:, :],
                            lhsT=XC[k // 4].rearrange(
                                "p (c t) -> p c t", c=4)[
                                :, c2 * 2:(c2 + 1) * 2,
                                (k % 4) * 128:(k % 4) * 128 + 128],
                            rhs=WV4[:, c2 * 1024:(c2 + 1) * 1024]
                                .rearrange("p (i n) -> p i n", i=2),
                            start=(c2 == 0), stop=(c2 == 1),
                            perf_mode=DR)

                    _drain_rr[0] = (_drain_rr[0] + 1) % 4
                    on_act = _drain_rr[0] != 0

                    def drain():
                        i = k % 2
                        dst = VB2[k // 2][:, i * 520:(i + 1) * 520].rearrange(
                            "p (h e) -> p h e", e=65)[:, :, 0:64]
                        srcp = fps.rearrange("p (h e) -> p h e", e=64)
                        if on_act:
                            nc.scalar.copy(dst, srcp)
                        else:
                            nc.vector.tensor_copy(dst, srcp)
                    return drain
                return emit

            # ---- prelude: just enough for pair 0 step 0 ----
            kq_quantum(KT[0], WK, 0, 0, "FA", act_copy=True)()()
            kq_quantum(QT[0], WQ, 0, 0, "SA")()()
            kq_quantum(QT[0], WQ, 0, 1, "DA", act_copy=True)()()
            done.update({("K", 0, 0), ("Q", 0, 0), ("Q", 0, 1)})

            # ---- feeder queue, ordered by earliest-deadline (global step
            # at which the quantum's output is first consumed) so pump(1)
            # per step meets every deadline with uniform drain pacing ----
            fq = []
            for k in range(1, KC):
                fq.append((2 * k + 7, ("V", k), v_quantum(k)))
            fq.append((7, ("V", 0), v_quantum(0)))
            for j in range(1, 4):
                fq.append((4 * j, ("K", 0, j), kq_quantum(KT[0], WK, 0, j)))
            for m in range(1, 4):
                fq.append((16 * m - 1, ("K", m, 0),
                           kq_quantum(KT[m], WK, m, 0)))
                for j in range(2):
                    fq.append((16 * m - 1, ("Q", m, j),
                               kq_quantum(QT[m], WQ, m, j)))
                for j in range(1, 4):
                    fq.append((16 * m + 4 * j, ("K", m, j),
                               kq_quantum(KT[m], WK, m, j)))
            fq.sort(key=lambda e: e[0])
            feeders.extend((key, fn) for _, key, fn in fq)

            # deferred epilogue inputs
            WO2 = cst.tile_from(wo[:, :], name="WO2")   # [128, pair*512+cout]
            XS = [cst.tile_from(xseq[t * 128:(t + 1) * 128, :], name=f"XS{t}")
                  for t in range(8)]
            GM = cst.tile_from(gmm[:, :], name="GM")
            BT = cst.tile_from(bet[:, :], name="BT")
            epsT = cst.tile([128, 1], F32, name="epsT")
            nc.vector.memset(epsT[:, :], EPS)

            # ---- attention ----
            def make_tp_quantum(p, ON, half):
                def emit():
                    tp = ps.tile([128, 512], BF16, tag="FA",
                                 name=f"tp{p}_{half}")
                    for j in range(4):
                        s = half * 4 + j
                        nc.tensor.transpose(
                            tp[:, j * 128:(j + 1) * 128],
                            ON[:, s * 128:(s + 1) * 128], ID[:, :])
                    if half == 0:
                        nc.vector.tensor_copy(
                            OT[:, p * TQ:p * TQ + 512], tp[:, :])
                    else:
                        nc.scalar.copy(
                            OT[:, p * TQ + 512:p * TQ + 1024], tp[:, :])
                return emit

            def make_boundary(p, oc):
                # normalize pair p: reciprocal of denominators, then one
                # broadcast tensor_tensor per contiguous oc-block group.
                # ON layout: [128, s*128 + hi*64] (s-major) so the
                # transposes read plain 128-col slices.
                def emit():
                    rd16 = sml.tile([128, 16], F32, tag="rd", name=f"rd{p}",
                                    bufs=2)
                    for lo, nblk, b0 in ((0, 7, 0), (512, 7, 7), (1024, 2, 14)):
                        nc.vector.reciprocal(
                            rd16[:, b0:b0 + nblk].rearrange(
                                "p (s e) -> p s e", e=1),
                            oc[:, lo:lo + nblk * 65].rearrange(
                                "p (s e) -> p s e", e=65)[:, :, 64:65])
                    ON = sml.tile([128, 1024], BF16, tag="on",
                                  name=f"on{p}", bufs=2)
                    ONb = ON.rearrange("p (q e) -> p q e", e=64)
                    # groups of oc blocks b = hi*8+s with uniform strides:
                    # (b0..b0+n) -> ON block index s*2+hi
                    for b0, nblk, lo in ((0, 7, 0), (7, 1, 512),
                                         (8, 6, 512 + 65), (14, 2, 1024)):
                        hi, s0 = divmod(b0, 8)
                        nc.vector.tensor_tensor(
                            out=ONb[:, 2 * s0 + hi:2 * (s0 + nblk - 1) + hi + 1:2, :],
                            in0=oc[:, lo:lo + nblk * 65].rearrange(
                                "p (s e) -> p s e", e=65)[:, :, 0:64],
                            in1=rd16[:, b0:b0 + nblk].rearrange(
                                "p (s e) -> p s e", e=1).to_broadcast(
                                [128, nblk, 64]),
                            op=MULT)
                    tp_pending.append((("TP", p, 0), make_tp_quantum(p, ON, 0)))
                    tp_pending.append((("TP", p, 1), make_tp_quantum(p, ON, 1)))
                return emit

            pending_boundary = None
            for p in range(PAIRS):
                oc = ps.tile([128, 1536], F32, tag="OC", name=f"oc{p}")

                def pv_emit(kk, pts, oc=oc, p=p):
                    for hi in range(2):
                        head = 2 * p + hi
                        pt8, ptb = pts[hi]
                        rhs2 = VB2[kk].rearrange("p (i c) -> p i c", i=2)[
                            :, :, head * 65:(head + 1) * 65]
                        for s in range(4):
                            nc.tensor.matmul(
                                oc[:, _bcol(hi * 8 + s):_bcol(hi * 8 + s) + 65],
                                lhsT=pt8.rearrange("p (i q) -> p i q", i=2)[
                                    :, :, s * 128:(s + 1) * 128],
                                rhs=rhs2,
                                start=(kk == 0), stop=(kk == KK - 1),
                                perf_mode=DR)
                        for s in range(4, 8):
                            col = _bcol(hi * 8 + s)
                            q0 = (s - 4) * 128
                            for i in range(2):
                                nc.tensor.matmul(
                                    oc[:, col:col + 65],
                                    lhsT=ptb[:, i * NA + q0:
                                             i * NA + q0 + 128].bitcast(BF16),
                                    rhs=VB2[kk][:, i * 520 + head * 65:
                                                i * 520 + (head + 1) * 65],
                                    start=(kk == 0 and i == 0),
                                    stop=(kk == KK - 1 and i == 1))

                ensure(("Q", p, 0))
                ensure(("Q", p, 1))
                lag = []      # chunk-pair PV deferral
                pts_cur = {}
                for k in range(KC):
                    ensure(("K", p, k // 4))
                    kk = k // 2
                    if k % 2 == 0:
                        pts_cur = {
                            hi: (pex.tile([128, 2 * NA], FP8,
                                          tag=f"p8{hi}", name=f"p8_{p}_{kk}_{hi}"),
                                 pex.tile([128, 2 * NA], I16,
                                          tag=f"pb{hi}", name=f"pb_{p}_{kk}_{hi}"))
                            for hi in range(2)}
                    s_tiles = []
                    for hi in range(2):
                        rows = slice(hi * 64, (hi + 1) * 64)
                        sa = ps.tile([128, 512], F32, tag=("SA", "SB")[hi],
                                     name=f"s{p}_{k}_{hi}")
                        nc.tensor.matmul(
                            sa[:, :],
                            lhsT=KT[p][rows, k * 128:(k + 1) * 128],
                            rhs=QT[p][rows, 0:512],
                            start=True, stop=True)
                        s_tiles.append(sa)
                    for hi in range(2):
                        rows = slice(hi * 64, (hi + 1) * 64)
                        sd = ps.tile([128, 512], F32, tag=("DA", "DB")[hi],
                                     name=f"d{p}_{k}_{hi}")
                        nc.tensor.matmul(
                            sd[:, :],
                            lhsT=KT[p][rows, k * 128:(k + 1) * 128],
                            rhs=QT[p][rows, 512:1024],
                            start=True, stop=True)
                        s_tiles[hi] = (s_tiles[hi], sd)
                    for hi in range(2):
                        sa, sd = s_tiles[hi]
                        pt8, ptb = pts_cur[hi]
                        i = k % 2
                        nc.scalar.activation(
                            pt8[:, i * NA:(i + 1) * NA], sa[:, :], Exp,
                            bias=AB[:, k:k + 1], scale=1.0)
                        nc.vector.tensor_scalar(
                            out=ptb[:, i * NA:(i + 1) * NA],
                            in0=sd[:, :], scalar1=A16,
                            scalar2=SB16[:, k:k + 1], op0=MULT, op1=ADD)
                    if k == 0 and pending_boundary is not None:
                        pending_boundary()
                        pending_boundary = None
                    if k % 2 == 1:
                        lag.append((kk, pts_cur))
                    # drain the PV lag progressively near the pair end
                    depth = 3 if k < KC - 3 else (2 if k < KC - 1 else 1)
                    while len(lag) > depth:
                        kkd, pp = lag.pop(0)
                        ensure(("V", 2 * kkd))
                        ensure(("V", 2 * kkd + 1))
                        pv_emit(kkd, pp)
                    if k in (5, 6):
                        run_tp()
                    if k < KC - 2:
                        pump(1)
                for kkd, pp in lag:
                    ensure(("V", 2 * kkd))
                    ensure(("V", 2 * kkd + 1))
                    pv_emit(kkd, pp)
                pending_boundary = make_boundary(p, oc)

            pending_boundary()
            run_tp(2)

            # ---- tail: out-proj (fp8-free bf16 DR over pair planes) + LN ----
            nmrs, rstds, accs = [], [], {}

            def tail_o(t):
                ensure(("TP", 3, t // 4))
                tag = ("SA", "DA", "FA")[t % 3]
                fps = ps.tile([128, 512], F32, tag=tag, name=f"fo{t}")
                for j in range(2):
                    nc.tensor.matmul(
                        fps[:, :],
                        lhsT=OT.rearrange("p (q4 q) -> p q4 q", q4=4)[
                            :, 2 * j:2 * j + 2, t * 128:(t + 1) * 128],
                        rhs=WO2[:, j * 1024:(j + 1) * 1024]
                            .rearrange("p (i n) -> p i n", i=2),
                        start=(j == 0), stop=(j == 1),
                        perf_mode=DR)
                acc = sml.tile([128, 1], F32, tag="acc", name=f"acc{t}", bufs=4)
                accs[t] = acc
                nc.vector.scalar_tensor_tensor(
                    out=OACC[t], in0=fps[:, :], scalar=1.0 / 1024, in1=XS[t],
                    op0=MULT, op1=ADD, accum_out=acc)

            def tail_a(t):
                sq = sml.tile([128, C], F32, tag="sq", name=f"sq{t}", bufs=2)
                ssq = sml.tile([128, 1], F32, tag="ssq", name=f"ssq{t}", bufs=3)
                nc.scalar.activation(sq[:, :], OACC[t][:, :], Square,
                                     accum_out=ssq[:, :])
                dvar = sml.tile([128, 1], F32, tag="dvar", name=f"dv{t}", bufs=3)
                nc.gpsimd.tensor_scalar(out=dvar[:, :], in0=accs[t],
                                        scalar1=accs[t], scalar2=-1.0 / C,
                                        op0=MULT, op1=MULT)
                nc.vector.scalar_tensor_tensor(
                    out=dvar[:, :], in0=dvar[:, :], scalar=1.0,
                    in1=ssq[:, :], op0=MULT, op1=ADD)
                std = sml.tile([128, 1], F32, tag="std", name=f"std{t}", bufs=3)
                nc.scalar.activation(std[:, :], dvar[:, :], Sqrt,
                                     bias=epsT[:, :], scale=1.0 / C)
                rstd = sml.tile([128, 1], F32, tag="rstd", name=f"rstd{t}", bufs=3)
                nc.vector.reciprocal(rstd[:, :], std[:, :])
                nmr = sml.tile([128, 1], F32, tag="nmr", name=f"nmr{t}", bufs=3)
                nc.gpsimd.tensor_scalar(out=nmr[:, :], in0=accs[t],
                                        scalar1=rstd[:, :], scalar2=-1.0 / C,
                                        op0=MULT, op1=MULT)
                rstds.append(rstd)
                nmrs.append(nmr)

            def tail_b(t):
                hn = sml.tile([128, C], BF16, tag="hn", name=f"hn{t}", bufs=3)
                nc.gpsimd.tensor_scalar(out=hn[:, :], in0=OACC[t][:, :],
                                        scalar1=rstds[t], scalar2=nmrs[t],
                                        op0=MULT, op1=ADD)
                fa = ps.tile([128, 256], BF16, tag="OC", name=f"ftpa{t}")
                fb = ps.tile([128, 256], BF16, tag="FA", name=f"ftpb{t}")
                for cc in range(4):
                    dst = fa if cc < 2 else fb
                    nc.tensor.transpose(
                        dst[:, (cc % 2) * 128:(cc % 2) * 128 + 128],
                        hn[:, cc * 128:(cc + 1) * 128], ID[:, :])
                oa = sml.tile([128, 256], F32, tag="outa", name=f"outa{t}", bufs=2)
                ob = sml.tile([128, 256], F32, tag="outb", name=f"outb{t}", bufs=2)
                for cc in range(2):
                    nc.scalar.activation(
                        oa[:, cc * 128:(cc + 1) * 128],
                        fa[:, cc * 128:(cc + 1) * 128],
                        Ident, bias=BT[:, cc:cc + 1], scale=GM[:, cc:cc + 1])
                for cc in range(2, 4):
                    nc.vector.tensor_scalar(
                        out=ob[:, (cc - 2) * 128:(cc - 1) * 128],
                        in0=fb[:, (cc - 2) * 128:(cc - 1) * 128],
                        scalar1=GM[:, cc:cc + 1], scalar2=BT[:, cc:cc + 1],
                        op0=MULT, op1=ADD)
                nc.sync.dma_start(
                    out=outp[0:256, t * 128:(t + 1) * 128].rearrange(
                        "(c p) q -> p c q", p=128),
                    in_=oa.rearrange("p (c q) -> p c q", c=2))
                nc.sync.dma_start(
                    out=outp[256:512, t * 128:(t + 1) * 128].rearrange(
                        "(c p) q -> p c q", p=128),
                    in_=ob.rearrange("p (c q) -> p c q", c=2))

            for t in range(8):
                tail_o(t)
                tail_a(t)
                if t >= 1:
                    tail_b(t - 1)
            tail_b(7)

    _split_mm_waits(nc)
    return nc


def _split_mm_waits(nc):
    """Walrus MM structs carry only one sync wait; move extras to a NoOp."""
    f = nc.m.functions[0]
    for bb in f.blocks:
        il = bb.instructions
        out, changed = [], False
        for i in il:
            si = getattr(i, "sync_info", None)
            tn = type(i).__name__
            splittable = tn.startswith("Inst") and tn not in ("InstNoOp", "InstAllEngineBarrier")
            if (splittable and si is not None
                    and si.on_wait is not None and len(si.on_wait) > 1):
                waits = list(si.on_wait)
                for wi, w in enumerate(waits[:-1]):
                    out.append(mybir.InstNoOp(
                        name=f"{i.name}-wsplit{wi}", engine=i.engine,
                        sync_info=mybir.SyncInfo(on_wait=[w], on_update=[])))
                i.sync_info = mybir.SyncInfo(
                    on_wait=[waits[-1]], on_update=list(si.on_update))
                changed = True
            out.append(i)
        if changed:
            bb.instructions = out


def _prep_inputs(x, sqi, w_qkv, w_out, b_out, w_conv, b_conv, ln_gamma, ln_beta):
    x = np.asarray(x, np.float32)
    sqi = np.asarray(sqi, np.float32)
    w_qkv = np.asarray(w_qkv, np.float32)
    w_out = np.asarray(w_out, np.float32)
    b_out = np.asarray(b_out, np.float32)
    w_conv = np.asarray(w_conv, np.float32)
    b_conv = np.asarray(b_conv, np.float32)
    ln_gamma = np.asarray(ln_gamma, np.float32)
    ln_beta = np.asarray(ln_beta, np.float32)

    sp = np.pad(sqi, ((0, 0), (1, 1)))
    bias = (w_conv[0] * sp[:, :-2] + w_conv[1] * sp[:, 1:-1]
            + w_conv[2] * sp[:, 2:] + b_conv)                    # (B, T)

    def pack4(a):
        """(512, n) -> [128, 4*n]: 128-row blocks side by side (one DMA)."""
        n = a.shape[1]
        return a.reshape(4, 128, n).transpose(1, 0, 2).reshape(128, 4 * n)

    def pack_mm(a):
        """(512, 512) -> [128, m*512 + ci*128 + c]: m-major so the m=0
        quarter is a contiguous prefix (separately-DMA'd tile)."""
        return a.reshape(4, 128, 4, 128).transpose(1, 2, 0, 3).reshape(128, 2048)

    wqT = pack_mm(w_qkv[:C].T * SCALE).astype(fp8)
    wkT = pack_mm(w_qkv[C:2 * C].T).astype(fp8)
    wvT = pack4(w_qkv[2 * C:].T).astype(fp8)
    woT = pack4(w_out.T * 16.0).astype(fp8)
    gm = ln_gamma.reshape(4, 128).T.copy().astype(np.float32)
    bt = ln_beta.reshape(4, 128).T.copy().astype(np.float32)
    iden = np.eye(128, dtype=bf16)

    in_maps = []
    for core in range(8):
        b, qh = divmod(core, 2)
        qs = slice(qh * TQ, (qh + 1) * TQ)
        # rotate tokens so this core's query half is chunks j=0,1
        xr = np.roll(x[b], -qh * TQ, axis=1)
        br = np.roll(bias[b], -qh * TQ)
        ab = br.reshape(KC, 128).T.copy().astype(np.float32)
        sb = (128.0 * (127.0 + LOG2E * br) + C_ADJ).reshape(
            KC, 128).T.copy().astype(np.float32)
        xp = np.concatenate(
            [pack4(xr[:, j * 512:(j + 1) * 512]) for j in range(4)], axis=1)
        in_maps.append({
            "xct": xp.astype(fp8),
            "xseq": (x[b].T[qs] + b_out).copy().astype(np.float32),
            "wq": wqT, "wk": wkT, "wv": wvT, "wo": woT,
            "abia": ab, "sbia": sb, "gmm": gm, "bet": bt, "iden": iden,
        })
    return in_maps


def kernel(x, sqi, w_qkv, w_out, b_out, w_conv, b_conv, ln_gamma, ln_beta,
           _trace=False):
    if "nc" not in _CACHE:
        _CACHE["nc"] = _build_nc()
    nc = _CACHE["nc"]
    in_maps = _prep_inputs(x, sqi, w_qkv, w_out, b_out, w_conv, b_conv,
                           ln_gamma, ln_beta)
    res = run_bass_kernel_spmd(nc, in_maps, core_ids=list(range(8)), trace=_trace)
    _CACHE["last_result"] = res
    out = np.empty((B, C, T), np.float32)
    for core in range(8):
        b, qh = divmod(core, 2)
        out[b][:, qh * TQ:(qh + 1) * TQ] = res.results[core]["out"]
    return out


# revision 28
# speedup vs baseline: 1.2633x; 1.0089x over previous
"""Trainium2 Bass kernel for LogitBiasedSelfAttention1D.

Sharding: 8 cores = (batch b in 0..3) x (query half qh in 0..1).
Each core computes full attention (all 8 heads, all 2048 keys) for the
1024 queries of its batch half. No collectives.

Math decomposition (exactly equivalent to the reference up to fp):
  - conv1d key bias folded into exp:  softmax(S + bias) via the Act
    engine's per-partition activation bias (bias = key-indexed AP), and
    on the DVE via the Schraudolph bits constant.  V carries a 65th
    all-ones column per head so PV also produces the softmax
    denominators.
  - SCALE folded into w_q on host.
  - b_out + residual x_seq folded into one host-prepared addend.
  - LayerNorm gamma/beta folded into the final transpose drain.

Engines:
  - PE: all matmuls.  QKV / out-proj use fp8e4 DoubleRow (2 contraction
    planes per pass); PV uses fp8-DR for the Act-exp'd query columns and
    plain bf16 for the DVE-exp'd columns.  S stays bf16.
  - Act: exact exp (fp8e4 out) for NA of the 1024 query columns per
    (chunk, head-pair-half).
  - DVE: Schraudolph int16-bits exp (bf16 via bitcast) for the rest,
    plus all PSUM drains, PV normalize (broadcast tensor_tensor), and
    the out-proj accumulate.
  - Pool: LN tail scalar work.
"""

import sys

for _p in ("/opt/trn_rl_repo", "/root/.axon_site/_ro/trn_rl_repo"):
    if _p not in sys.path:
        sys.path.insert(0, _p)

import numpy as np
import ml_dtypes

from concourse import bass, mybir
from concourse.tile import TileContext
from concourse.bass_utils import run_bass_kernel_spmd

B, C, T = 4, 512, 2048
H, D = 8, 64
SCALE = D ** -0.5
EPS = 1e-5
TQ = T // 2            # queries per core
KC = T // 128          # 16 key chunks
KK = KC // 2           # 8 chunk pairs
PAIRS = H // 2         # 4 head pairs
F32 = mybir.dt.float32
BF16 = mybir.dt.bfloat16
FP8 = mybir.dt.float8e4
I16 = mybir.dt.int16
bf16 = ml_dtypes.bfloat16
fp8 = ml_dtypes.float8_e4m3

Exp = mybir.ActivationFunctionType.Exp
Sqrt = mybir.ActivationFunctionType.Sqrt
Square = mybir.ActivationFunctionType.Square
Ident = mybir.ActivationFunctionType.Identity
MULT = mybir.AluOpType.mult
ADD = mybir.AluOpType.add
DR = mybir.MatmulPerfMode.DoubleRow

LOG2E = 1.4426950408889634
A16 = 128.0 * LOG2E          # Schraudolph slope (bf16 bits)
C_ADJ = -128.0 * 0.04305     # balanced max-rel-err constant

# Query-column split: the n=0 half of each S tile is exp'd by Act, the
# n=1 half by the DVE (Schraudolph).  Separate PSUM tiles per engine --
# the tile framework serializes cross-engine co-readers of one tile.
NA = 512

_CACHE = {}


def _bcol(b):
    """Column offset of 65-wide PV block b (0..15) in the 3-bank OC tile.
    7 + 7 + 2 blocks per bank; no block crosses a 512-col bank boundary.
    b = hi*8 + s."""
    if b < 7:
        return b * 65
    if b < 14:
        return 512 + (b - 7) * 65
    return 1024 + (b - 14) * 65


def _build_nc():
    nc = bass.Bass()
    # packed layouts: one DMA per logical tensor; [128, n*512] with the
    # 128-row blocks of the original (rows, cols) tensor side by side.
    # Token chunks are rotated per core so this core's query half is always
    # chunks j=0,1 (softmax is key-order invariant; the per-key bias is
    # rotated to match), so Q-projection reads XC directly.
    xct = nc.declare_dram_parameter("xct", [128, 4 * T], FP8, False)
    xseq = nc.declare_dram_parameter("xseq", [TQ, C], F32, False)
    wq = nc.declare_dram_parameter("wq", [128, 4 * C], FP8, False)
    wk = nc.declare_dram_parameter("wk", [128, 4 * C], FP8, False)
    wv = nc.declare_dram_parameter("wv", [128, 4 * C], FP8, False)
    wo = nc.declare_dram_parameter("wo", [128, 4 * C], FP8, False)
    abia = nc.declare_dram_parameter("abia", [128, KC], F32, False)
    sbia = nc.declare_dram_parameter("sbia", [128, KC], F32, False)
    gmm = nc.declare_dram_parameter("gmm", [128, 4], F32, False)
    bet = nc.declare_dram_parameter("bet", [128, 4], F32, False)
    iden = nc.declare_dram_parameter("iden", [128, 128], BF16, False)
    outp = nc.declare_dram_parameter("out", [C, TQ], F32, True)

    with TileContext(nc) as tc:
        with (
            tc.sbuf_pool(name="cst", bufs=1) as cst,
            tc.sbuf_pool(name="pex", bufs=5) as pex,
            tc.sbuf_pool(name="sml", bufs=2) as sml,
            tc.psum_pool(name="ps", bufs=1) as ps,
        ):
            # ---- critical-path constants, in DMA priority order ----
            ID = cst.tile_from(iden[:, :], name="ID")
            WKa = cst.tile_from(wk[:, 0:512], name="WKa")
            XC = [None] * 4
            XC[0] = cst.tile_from(xct[:, 0:2048], name="XCj0")
            WQa = cst.tile_from(wq[:, 0:512], name="WQa")
            XC[1] = cst.tile_from(xct[:, 2048:4096], name="XCj1")
            WKb = cst.tile_from(wk[:, 512:2048], name="WKb")
            WQb = cst.tile_from(wq[:, 512:2048], name="WQb")
            AB = cst.tile_from(abia[:, :], name="AB")
            SB16 = cst.tile_from(sbia[:, :], name="SB16")
            WV4 = cst.tile_from(wv[:, :], name="WV4")
            for j in range(2, 4):
                XC[j] = cst.tile_from(xct[:, j * 2048:(j + 1) * 2048],
                                      name=f"XCj{j}")
            WK = (WKa, WKb)
            WQ = (WQa, WQb)

            # PE p-state warmup: chain dummy transposes while the first
            # input DMAs stream in.
            warm = ps.tile([128, 128], BF16, tag="FA", name="warm")
            for _ in range(48):
                nc.tensor.transpose(warm[:, :], ID[:, :], ID[:, :])

            # ---- persistent SBUF tiles ----
            KT = [cst.tile([128, T], BF16, name=f"KT{m}") for m in range(4)]
            QT = [cst.tile([128, TQ], BF16, name=f"QT{m}") for m in range(4)]
            # VB2[kk]: [128, 2*(H*65)] fp8, plane i = chunk 2kk+i; the 65th
            # column of each head block is 1.0 (softmax denominator).
            VB2 = [cst.tile([128, 2 * H * 65], FP8, name=f"VB{k}")
                   for k in range(KK)]
            OT = cst.tile([128, 4 * TQ], FP8, name="OT")   # [128, pair, TQ]
            OACC = [cst.tile([128, C], F32, name=f"OACC{t}") for t in range(8)]

            for kk in range(KK):
                for i in range(2):
                    nc.gpsimd.memset(
                        VB2[kk][:, i * 520:(i + 1) * 520].rearrange(
                            "p (h e) -> p h e", e=65)[:, :, 64:65], 1.0 / 64)

            # ---- feeder machinery ----
            # Each quantum's emit() issues its PE matmuls and returns a
            # drain closure (PSUM -> SBUF copy).  The drain is deferred
            # until the NEXT quantum is pumped, so by the time it lands on
            # the DVE queue its matmul has long finished and it never
            # head-of-line-blocks the critical exp TSPs.
            feeders = []
            done = set()
            pending = []   # (key, drain_fn), at most 1 entry
            tp_pending = []  # [(key, fn)] transpose quanta, run mid-pair

            def _flush():
                while pending:
                    key, dfn = pending.pop(0)
                    dfn()
                    done.add(key)

            def pump(n=1):
                for _ in range(n):
                    if feeders:
                        _flush()
                        key, fn = feeders.pop(0)
                        d = fn()
                        if d is None:
                            done.add(key)
                        else:
                            pending.append((key, d))

            def run_tp(n=1):
                for _ in range(n):
                    if tp_pending:
                        key, fn = tp_pending.pop(0)
                        fn()
                        done.add(key)

            def ensure(key):
                while key not in done:
                    if any(k2 == key for k2, _ in pending):
                        _flush()
                        continue
                    if any(k2 == key for k2, _ in tp_pending):
                        run_tp()
                        continue
                    assert feeders, f"missing feeder quantum {key}"
                    _flush()
                    k2, fn = feeders.pop(0)
                    d = fn()
                    if d is not None:
                        d()
                    done.add(k2)

            _drain_rr = [0]

            def kq_quantum(dst, Wab, m, j, tag="FA", act_copy=None):
                # dst[:, j*512:(j+1)*512] = W[:, m-block].T @ x-cols-j
                # fp8 DoubleRow over ci-plane pairs.
                def emit():
                    W = Wab[0] if m == 0 else Wab[1]
                    c0 = (0 if m == 0 else (m - 1) * 512)
                    fps = ps.tile([128, 512], F32, tag=tag,
                                  name=f"f_{dst.tensor.name}_{j}")
                    for c2 in range(2):
                        nc.tensor.matmul(
                            fps[:, :],
                            lhsT=W[:, c0 + c2 * 256:c0 + (c2 + 1) * 256]
                                .rearrange("p (i m2) -> p i m2", i=2),
                            rhs=XC[j][:, c2 * 1024:(c2 + 1) * 1024]
                                .rearrange("p (i n) -> p i n", i=2),
                            start=(c2 == 0), stop=(c2 == 1),
                            perf_mode=DR)

                    if act_copy is None:
                        _drain_rr[0] = (_drain_rr[0] + 1) % 4
                        on_act = _drain_rr[0] != 0
                    else:
                        on_act = act_copy

                    def drain():
                        if on_act:
                            nc.scalar.copy(dst[:, j * 512:(j + 1) * 512],
                                           fps[:, :])
                        else:
                            nc.vector.tensor_copy(
                                dst[:, j * 512:(j + 1) * 512], fps[:, :])
                    return drain
                return emit

            def v_quantum(k, tag="FA"):
                def emit():
                    fps = ps.tile([128, 512], F32, tag=tag, name=f"fv{k}")
                    for c2 in range(2):
                        nc.tensor.matmul(
                            fps[:, :],
                            lhsT=XC[k // 4].rearrange(
                                "p (c t) -> p c t", c=4)[
                                :, c2 * 2:(c2 + 1) * 2,
                                (k % 4) * 128:(k % 4) * 128 + 128],
                            rhs=WV4[:, c2 * 1024:(c2 + 1) * 1024]
                                .rearrange("p (i n) -> p i n", i=2),
                            start=(c2 == 0), stop=(c2 == 1),
                            perf_mode=DR)

                    _drain_rr[0] = (_drain_rr[0] + 1) % 4
                    on_act = _drain_rr[0] != 0

                    def drain():
                        i = k % 2
                        dst = VB2[k // 2][:, i * 520:(i + 1) * 520].rearrange(
                            "p (h e) -> p h e", e=65)[:, :, 0:64]
                        srcp = fps.rearrange("p (h e) -> p h e", e=64)
                        if on_act:
                            nc.scalar.copy(dst, srcp)
                        else:
                            nc.vector.tensor_copy(dst, srcp)
                    return drain
                return emit

            # ---- prelude: just enough for pair 0 step 0 ----
            kq_quantum(KT[0], WK, 0, 0, "FA", act_copy=True)()()
            kq_quantum(QT[0], WQ, 0, 0, "SA", act_copy=False)()()
            kq_quantum(QT[0], WQ, 0, 1, "DA", act_copy=True)()()
            done.update({("K", 0, 0), ("Q", 0, 0), ("Q", 0, 1)})

            # ---- feeder queue, ordered by earliest-deadline (global step
            # at which the quantum's output is first consumed) so pump(1)
            # per step meets every deadline with uniform drain pacing ----
            fq = []
            for k in range(1, KC):
                fq.append((2 * k + 7, ("V", k), v_quantum(k)))
            fq.append((7, ("V", 0), v_quantum(0)))
            for j in range(1, 4):
                fq.append((4 * j, ("K", 0, j), kq_quantum(KT[0], WK, 0, j)))
            for m in range(1, 4):
                fq.append((16 * m - 1, ("K", m, 0),
                           kq_quantum(KT[m], WK, m, 0)))
                for j in range(2):
                    fq.append((16 * m - 1, ("Q", m, j),
                               kq_quantum(QT[m], WQ, m, j)))
                for j in range(1, 4):
                    fq.append((16 * m + 4 * j, ("K", m, j),
                               kq_quantum(KT[m], WK, m, j)))
            fq.sort(key=lambda e: e[0])
            feeders.extend((key, fn) for _, key, fn in fq)

            # deferred epilogue inputs
            WO2 = cst.tile_from(wo[:, :], name="WO2")   # [128, pair*512+cout]
            XS = [cst.tile_from(xseq[t * 128:(t + 1) * 128, :], name=f"XS{t}")
                  for t in range(8)]
            GM = cst.tile_from(gmm[:, :], name="GM")
            BT = cst.tile_from(bet[:, :], name="BT")
            epsT = cst.tile([128, 1], F32, name="epsT")
            nc.vector.memset(epsT[:, :], EPS)

            # ---- attention ----
            def make_tp_quantum(p, ON, half):
                def emit():
                    tp = ps.tile([128, 512], BF16, tag="FA",
                                 name=f"tp{p}_{half}")
                    for j in range(4):
                        s = half * 4 + j
                        nc.tensor.transpose(
                            tp[:, j * 128:(j + 1) * 128],
                            ON[:, s * 128:(s + 1) * 128], ID[:, :])
                    if half == 0:
                        nc.vector.tensor_copy(
                            OT[:, p * TQ:p * TQ + 512], tp[:, :])
                    else:
                        nc.scalar.copy(
                            OT[:, p * TQ + 512:p * TQ + 1024], tp[:, :])
                return emit

            def make_boundary(p, oc):
                # normalize pair p: reciprocal of denominators, then one
                # broadcast tensor_tensor per contiguous oc-block group.
                # ON layout: [128, s*128 + hi*64] (s-major) so the
                # transposes read plain 128-col slices.
                def emit():
                    rd16 = sml.tile([128, 16], F32, tag="rd", name=f"rd{p}",
                                    bufs=2)
                    for lo, nblk, b0 in ((0, 7, 0), (512, 7, 7), (1024, 2, 14)):
                        nc.vector.reciprocal(
                            rd16[:, b0:b0 + nblk].rearrange(
                                "p (s e) -> p s e", e=1),
                            oc[:, lo:lo + nblk * 65].rearrange(
                                "p (s e) -> p s e", e=65)[:, :, 64:65])
                    ON = sml.tile([128, 1024], BF16, tag="on",
                                  name=f"on{p}", bufs=2)
                    ONb = ON.rearrange("p (q e) -> p q e", e=64)
                    # groups of oc blocks b = hi*8+s with uniform strides:
                    # (b0..b0+n) -> ON block index s*2+hi
                    for b0, nblk, lo in ((0, 7, 0), (7, 1, 512),
                                         (8, 6, 512 + 65), (14, 2, 1024)):
                        hi, s0 = divmod(b0, 8)
                        nc.vector.tensor_tensor(
                            out=ONb[:, 2 * s0 + hi:2 * (s0 + nblk - 1) + hi + 1:2, :],
                            in0=oc[:, lo:lo + nblk * 65].rearrange(
                                "p (s e) -> p s e", e=65)[:, :, 0:64],
                            in1=rd16[:, b0:b0 + nblk].rearrange(
                                "p (s e) -> p s e", e=1).to_broadcast(
                                [128, nblk, 64]),
                            op=MULT)
                    tp_pending.append((("TP", p, 0), make_tp_quantum(p, ON, 0)))
                    tp_pending.append((("TP", p, 1), make_tp_quantum(p, ON, 1)))
                return emit

            pending_boundary = None
            lag = []      # chunk-pair PV deferral, carried across pairs
            gstep = [0, 0]   # [global step, quanta pumped]
            for p in range(PAIRS):
                oc = ps.tile([128, 1536], F32, tag="OC", name=f"oc{p}")

                def pv_emit(kk, pts, oc=oc, p=p):
                    for hi in range(2):
                        head = 2 * p + hi
                        pt8, ptb, is8 = pts[hi]
                        rhs2 = VB2[kk].rearrange("p (i c) -> p i c", i=2)[
                            :, :, head * 65:(head + 1) * 65]
                        for s in range(4):
                            nc.tensor.matmul(
                                oc[:, _bcol(hi * 8 + s):_bcol(hi * 8 + s) + 65],
                                lhsT=pt8.rearrange("p (i q) -> p i q", i=2)[
                                    :, :, s * 128:(s + 1) * 128],
                                rhs=rhs2,
                                start=(kk == 0), stop=(kk == KK - 1),
                                perf_mode=DR)
                        if is8:
                            for s in range(4, 8):
                                nc.tensor.matmul(
                                    oc[:, _bcol(hi * 8 + s):
                                       _bcol(hi * 8 + s) + 65],
                                    lhsT=ptb.rearrange(
                                        "p (i q) -> p i q", i=2)[
                                        :, :, (s - 4) * 128:(s - 3) * 128],
                                    rhs=rhs2,
                                    start=(kk == 0), stop=(kk == KK - 1),
                                    perf_mode=DR)
                        else:
                            for s in range(4, 8):
                                col = _bcol(hi * 8 + s)
                                q0 = (s - 4) * 128
                                for i in range(2):
                                    nc.tensor.matmul(
                                        oc[:, col:col + 65],
                                        lhsT=ptb[:, i * NA + q0:
                                                 i * NA + q0 + 128].bitcast(BF16),
                                        rhs=VB2[kk][:, i * 520 + head * 65:
                                                    i * 520 + (head + 1) * 65],
                                        start=(kk == 0 and i == 0),
                                        stop=(kk == KK - 1 and i == 1))

                ensure(("Q", p, 0))
                ensure(("Q", p, 1))
                pts_cur = {}
                for k in range(KC):
                    ensure(("K", p, k // 4))
                    kk = k // 2
                    act_d = (p == PAIRS - 1 and k >= 14)
                    if k % 2 == 0:
                        pts_cur = {
                            hi: (pex.tile([128, 2 * NA], FP8,
                                          tag=f"p8{hi}", name=f"p8_{p}_{kk}_{hi}"),
                                 pex.tile([128, 2 * NA], FP8 if act_d else I16,
                                          tag=(f"pc{hi}" if act_d else f"pb{hi}"),
                                          name=f"pb_{p}_{kk}_{hi}"),
                                 act_d)
                            for hi in range(2)}
                    s_tiles = []
                    for hi in range(2):
                        rows = slice(hi * 64, (hi + 1) * 64)
                        sa = ps.tile([128, 512], F32, tag=("SA", "SB")[hi],
                                     name=f"s{p}_{k}_{hi}")
                        nc.tensor.matmul(
                            sa[:, :],
                            lhsT=KT[p][rows, k * 128:(k + 1) * 128],
                            rhs=QT[p][rows, 0:512],
                            start=True, stop=True)
                        s_tiles.append(sa)
                    for hi in range(2):
                        rows = slice(hi * 64, (hi + 1) * 64)
                        sd = ps.tile([128, 512], F32, tag=("DA", "DB")[hi],
                                     name=f"d{p}_{k}_{hi}")
                        nc.tensor.matmul(
                            sd[:, :],
                            lhsT=KT[p][rows, k * 128:(k + 1) * 128],
                            rhs=QT[p][rows, 512:1024],
                            start=True, stop=True)
                        s_tiles[hi] = (s_tiles[hi], sd)
                    for hi in range(2):
                        sa, sd = s_tiles[hi]
                        pt8, ptb, is8 = pts_cur[hi]
                        i = k % 2
                        nc.scalar.activation(
                            pt8[:, i * NA:(i + 1) * NA], sa[:, :], Exp,
                            bias=AB[:, k:k + 1], scale=1.0)
                        if is8:
                            nc.scalar.activation(
                                ptb[:, i * NA:(i + 1) * NA], sd[:, :], Exp,
                                bias=AB[:, k:k + 1], scale=1.0)
                        else:
                            nc.vector.tensor_scalar(
                                out=ptb[:, i * NA:(i + 1) * NA],
                                in0=sd[:, :], scalar1=A16,
                                scalar2=SB16[:, k:k + 1], op0=MULT, op1=ADD)
                    if k == 1 and pending_boundary is not None:
                        # finish the previous pair's deferred PV, then its
                        # normalize -- AFTER this pair's first S/exp groups
                        # so the next-pair pipeline never queues behind them.
                        while lag:
                            kkd, pp, pvf = lag.pop(0)
                            ensure(("V", 2 * kkd))
                            ensure(("V", 2 * kkd + 1))
                            pvf(kkd, pp)
                        pending_boundary()
                        pending_boundary = None
                    if k % 2 == 1:
                        lag.append((kk, pts_cur, pv_emit))
                    while len(lag) > 3:
                        kkd, pp, pvf = lag.pop(0)
                        ensure(("V", 2 * kkd))
                        ensure(("V", 2 * kkd + 1))
                        pvf(kkd, pp)
                    if k in (5, 6):
                        run_tp()
                    if k < KC - 2:
                        pump(1)
                pending_boundary = make_boundary(p, oc)

            while lag:
                kkd, pp, pvf = lag.pop(0)
                ensure(("V", 2 * kkd))
                ensure(("V", 2 * kkd + 1))
                pvf(kkd, pp)
            pending_boundary()
            run_tp(2)

            # ---- tail: out-proj (fp8-free bf16 DR over pair planes) + LN ----
            nmrs, rstds, accs = [], [], {}

            def tail_o(t):
                ensure(("TP", 3, t // 4))
                tag = ("SA", "DA", "FA")[t % 3]
                fps = ps.tile([128, 512], F32, tag=tag, name=f"fo{t}")
                for j in range(2):
                    nc.tensor.matmul(
                        fps[:, :],
                        lhsT=OT.rearrange("p (q4 q) -> p q4 q", q4=4)[
                            :, 2 * j:2 * j + 2, t * 128:(t + 1) * 128],
                        rhs=WO2[:, j * 1024:(j + 1) * 1024]
                            .rearrange("p (i n) -> p i n", i=2),
                        start=(j == 0), stop=(j == 1),
                        perf_mode=DR)
                acc = sml.tile([128, 1], F32, tag="acc", name=f"acc{t}", bufs=4)
                accs[t] = acc
                nc.vector.scalar_tensor_tensor(
                    out=OACC[t], in0=fps[:, :], scalar=1.0 / 1024, in1=XS[t],
                    op0=MULT, op1=ADD, accum_out=acc)

            def tail_a(t):
                sq = sml.tile([128, C], F32, tag="sq", name=f"sq{t}", bufs=2)
                ssq = sml.tile([128, 1], F32, tag="ssq", name=f"ssq{t}", bufs=3)
                nc.scalar.activation(sq[:, :], OACC[t][:, :], Square,
                                     accum_out=ssq[:, :])
                dvar = sml.tile([128, 1], F32, tag="dvar", name=f"dv{t}", bufs=3)
                nc.gpsimd.tensor_scalar(out=dvar[:, :], in0=accs[t],
                                        scalar1=accs[t], scalar2=-1.0 / C,
                                        op0=MULT, op1=MULT)
                nc.vector.scalar_tensor_tensor(
                    out=dvar[:, :], in0=dvar[:, :], scalar=1.0,
                    in1=ssq[:, :], op0=MULT, op1=ADD)
                std = sml.tile([128, 1], F32, tag="std", name=f"std{t}", bufs=3)
                nc.scalar.activation(std[:, :], dvar[:, :], Sqrt,
                                     bias=epsT[:, :], scale=1.0 / C)
                rstd = sml.tile([128, 1], F32, tag="rstd", name=f"rstd{t}", bufs=3)
                nc.vector.reciprocal(rstd[:, :], std[:, :])
                nmr = sml.tile([128, 1], F32, tag="nmr", name=f"nmr{t}", bufs=3)
                nc.gpsimd.tensor_scalar(out=nmr[:, :], in0=accs[t],
                                        scalar1=rstd[:, :], scalar2=-1.0 / C,
                                        op0=MULT, op1=MULT)
                rstds.append(rstd)
                nmrs.append(nmr)

            def tail_b(t):
                hn = sml.tile([128, C], BF16, tag="hn", name=f"hn{t}", bufs=3)
                nc.gpsimd.tensor_scalar(out=hn[:, :], in0=OACC[t][:, :],
                                        scalar1=rstds[t], scalar2=nmrs[t],
                                        op0=MULT, op1=ADD)
                fa = ps.tile([128, 256], BF16, tag="OC", name=f"ftpa{t}")
                fb = ps.tile([128, 256], BF16, tag="FA", name=f"ftpb{t}")
                for cc in range(4):
                    dst = fa if cc < 2 else fb
                    nc.tensor.transpose(
                        dst[:, (cc % 2) * 128:(cc % 2) * 128 + 128],
                        hn[:, cc * 128:(cc + 1) * 128], ID[:, :])
                oa = sml.tile([128, 256], F32, tag="outa", name=f"outa{t}", bufs=2)
                ob = sml.tile([128, 256], F32, tag="outb", name=f"outb{t}", bufs=2)
                for cc in range(2):
                    nc.scalar.activation(
                        oa[:, cc * 128:(cc + 1) * 128],
                        fa[:, cc * 128:(cc + 1) * 128],
                        Ident, bias=BT[:, cc:cc + 1], scale=GM[:, cc:cc + 1])
                for cc in range(2, 4):
                    nc.vector.tensor_scalar(
                        out=ob[:, (cc - 2) * 128:(cc - 1) * 128],
                        in0=fb[:, (cc - 2) * 128:(cc - 1) * 128],
                        scalar1=GM[:, cc:cc + 1], scalar2=BT[:, cc:cc + 1],
                        op0=MULT, op1=ADD)
                nc.sync.dma_start(
                    out=outp[0:256, t * 128:(t + 1) * 128].rearrange(
                        "(c p) q -> p c q", p=128),
                    in_=oa.rearrange("p (c q) -> p c q", c=2))
                nc.sync.dma_start(
                    out=outp[256:512, t * 128:(t + 1) * 128].rearrange(
                        "(c p) q -> p c q", p=128),
                    in_=ob.rearrange("p (c q) -> p c q", c=2))

            for t in range(8):
                tail_o(t)
                tail_a(t)
                if t >= 1:
                    tail_b(t - 1)
            tail_b(7)

    _split_mm_waits(nc)
    return nc


def _split_mm_waits(nc):
    """Walrus MM structs carry only one sync wait; move extras to a NoOp."""
    f = nc.m.functions[0]
    for bb in f.blocks:
        il = bb.instructions
        out, changed = [], False
        for i in il:
            si = getattr(i, "sync_info", None)
            tn = type(i).__name__
            splittable = tn.startswith("Inst") and tn not in ("InstNoOp", "InstAllEngineBarrier")
            if (splittable and si is not None
                    and si.on_wait is not None and len(si.on_wait) > 1):
                waits = list(si.on_wait)
                for wi, w in enumerate(waits[:-1]):
                    out.append(mybir.InstNoOp(
                        name=f"{i.name}-wsplit{wi}", engine=i.engine,
                        sync_info=mybir.SyncInfo(on_wait=[w], on_update=[])))
                i.sync_info = mybir.SyncInfo(
                    on_wait=[waits[-1]], on_update=list(si.on_update))
                changed = True
            out.append(i)
        if changed:
            bb.instructions = out


def _prep_inputs(x, sqi, w_qkv, w_out, b_out, w_conv, b_conv, ln_gamma, ln_beta):
    x = np.asarray(x, np.float32)
    sqi = np.asarray(sqi, np.float32)
    w_qkv = np.asarray(w_qkv, np.float32)
    w_out = np.asarray(w_out, np.float32)
    b_out = np.asarray(b_out, np.float32)
    w_conv = np.asarray(w_conv, np.float32)
    b_conv = np.asarray(b_conv, np.float32)
    ln_gamma = np.asarray(ln_gamma, np.float32)
    ln_beta = np.asarray(ln_beta, np.float32)

    sp = np.pad(sqi, ((0, 0), (1, 1)))
    bias = (w_conv[0] * sp[:, :-2] + w_conv[1] * sp[:, 1:-1]
            + w_conv[2] * sp[:, 2:] + b_conv)                    # (B, T)

    def pack4(a):
        """(512, n) -> [128, 4*n]: 128-row blocks side by side (one DMA)."""
        n = a.shape[1]
        return a.reshape(4, 128, n).transpose(1, 0, 2).reshape(128, 4 * n)

    def pack_mm(a):
        """(512, 512) -> [128, m*512 + ci*128 + c]: m-major so the m=0
        quarter is a contiguous prefix (separately-DMA'd tile)."""
        return a.reshape(4, 128, 4, 128).transpose(1, 2, 0, 3).reshape(128, 2048)

    wqT = pack_mm(w_qkv[:C].T * SCALE).astype(fp8)
    wkT = pack_mm(w_qkv[C:2 * C].T).astype(fp8)
    wvT = pack4(w_qkv[2 * C:].T).astype(fp8)
    woT = pack4(w_out.T * 16.0).astype(fp8)
    gm = ln_gamma.reshape(4, 128).T.copy().astype(np.float32)
    bt = ln_beta.reshape(4, 128).T.copy().astype(np.float32)
    iden = np.eye(128, dtype=bf16)

    in_maps = []
    for core in range(8):
        b, qh = divmod(core, 2)
        qs = slice(qh * TQ, (qh + 1) * TQ)
        # rotate tokens so this core's query half is chunks j=0,1
        xr = np.roll(x[b], -qh * TQ, axis=1)
        br = np.roll(bias[b], -qh * TQ)
        ab = br.reshape(KC, 128).T.copy().astype(np.float32)
        sb = (128.0 * (127.0 + LOG2E * br) + C_ADJ).reshape(
            KC, 128).T.copy().astype(np.float32)
        xp = np.concatenate(
            [pack4(xr[:, j * 512:(j + 1) * 512]) for j in range(4)], axis=1)
        in_maps.append({
            "xct": xp.astype(fp8),
            "xseq": (x[b].T[qs] + b_out).copy().astype(np.float32),
            "wq": wqT, "wk": wkT, "wv": wvT, "wo": woT,
            "abia": ab, "sbia": sb, "gmm": gm, "bet": bt, "iden": iden,
        })
    return in_maps


def kernel(x, sqi, w_qkv, w_out, b_out, w_conv, b_conv, ln_gamma, ln_beta,
           _trace=False):
    if "nc" not in _CACHE:
        _CACHE["nc"] = _build_nc()
    nc = _CACHE["nc"]
    in_maps = _prep_inputs(x, sqi, w_qkv, w_out, b_out, w_conv, b_conv,
                           ln_gamma, ln_beta)
    res = run_bass_kernel_spmd(nc, in_maps, core_ids=list(range(8)), trace=_trace)
    _CACHE["last_result"] = res
    out = np.empty((B, C, T), np.float32)
    for core in range(8):
        b, qh = divmod(core, 2)
        out[b][:, qh * TQ:(qh + 1) * TQ] = res.results[core]["out"]
    return out
